# revision 2
# baseline (speedup 1.0000x reference)
"""Distributed attention kernel for Trainium2 (8 NeuronCores).

Problem: non-causal multi-head attention with GQA (16 q heads, 4 kv heads,
head_dim 64, dim 1024, batch 2, seqlen 2048), fp32.

Sharding (per the batch+head hint): core c in 0..7 handles batch b = c//4
and kv-head-group g = c%4 (q heads 4g..4g+3, kv head g). Each core holds the
full sequence, so softmax needs no communication. The output projection is
row-parallel: core (b, g) computes the partial product
O_g @ wo[256g:256(g+1), :] and the host sums the 4 partials per batch
(the gather/unshard step).

Per-core dataflow (v2 — PV restructured to seq-major output):
  xT = x[b].T                               (1024, S) fed from host, bf16
  QT = wq_g.T @ xT                          (256, S)  f32r [head pair ft:
                                              rows 0-63 head 2ft, 64-127 2ft+1]
  KVT = [wk_g | wv_g].T @ xT                (128, S)  f32r [K^T ; V^T]
  K^T duplicated to partitions 64-127 (gpsimd DMA) so both heads of a pair
  run score matmuls from disjoint partition ranges.
  V transposed per 128-k tile (PE) and packed seq-major with a ones column:
  v1[kt] = [V_kt | 1]  (128, 65) bf16.
  Per (qc of 512 q, ft head-pair):
    per kt: S^T = K^T.T @ Q^T -> psum [128, 2, 512]; one exp (ScalarE)
            -> e2t[:, :, kt, :] bf16 (slab for the whole phase).
    PV with the probabilities STATIONARY: out[q, d] += e2^T @ [V|1]
    accumulated qt-major into [128, 4, 65] psum (sequential sub-bank
    accumulation groups; hardware allows only one OPEN group per bank).
    Cost: 65 cols/moving pass instead of 512 -> halves PV PE time.
    Normalize: DVE reciprocal of the ones-column sum, GPSIMD per-partition
    multiply -> o bf16 [q, 2*64]; XBAR dma transpose -> ot feature-major.
  out rows = OT.T @ wo_g (PE), staged bf16, DMA out. Host sums partials.

Engine budget (cost model): ACT exp 128x~1.04us = 133us (bound), PE ~287K
cyc = 120us, DVE ~43us, Pool ~12us, DMA ~28us.
"""

import numpy as np
from contextlib import ExitStack

import concourse.bass as bass
import concourse.mybir as mybir
import concourse.tile as tile
from concourse.bass import ds
from concourse import bass_utils
from concourse.alu_op_type import AluOpType

F32 = mybir.dt.float32
F32R = mybir.dt.float32r
BF16 = mybir.dt.bfloat16

DIM = 1024
N_HEADS = 16
N_KV_HEADS = 4
HD = 64
FH = 256                   # q features per core (4 heads x 64)
KV = 128                   # [K | V] projected feature width per core
D_TILES = DIM // 128       # 8
SEQ = 2048
BSZ = 2
N_CORES = 8


def build_attention_core(nc, S=SEQ, use_f32r=True):
    """Emit the per-core kernel. S = sequence length (multiple of 512)."""
    QCH = 512                  # q-chunk width (psum bank = 512 f32)
    S_TILES = S // 128         # k tiles
    N_QC = S // QCH            # q chunks
    MDT = F32R if use_f32r else F32

    xT = nc.declare_dram_parameter("xT", [DIM, S], BF16, isOutput=False)
    identd = nc.declare_dram_parameter("ident", [128, 128], MDT, isOutput=False)
    wq = nc.declare_dram_parameter("wq", [DIM, FH], BF16, isOutput=False)
    wkv = nc.declare_dram_parameter("wkv", [DIM, KV], BF16, isOutput=False)
    wo = nc.declare_dram_parameter("wo", [FH, DIM], BF16, isOutput=False)
    out = nc.declare_dram_parameter("out", [S, DIM], BF16, isOutput=True)

    with tile.TileContext(nc) as tc:
      with ExitStack() as ctx:
        const_p = ctx.enter_context(tc.tile_pool(name="const", bufs=1))
        big_p = ctx.enter_context(tc.tile_pool(name="big", bufs=1))
        e2_p = ctx.enter_context(tc.tile_pool(name="e2", bufs=2))
        of_p = ctx.enter_context(tc.tile_pool(name="of", bufs=2))
        osb_p = ctx.enter_context(tc.tile_pool(name="osb", bufs=2))
        rz_p = ctx.enter_context(tc.tile_pool(name="rz", bufs=8))
        stg_p = ctx.enter_context(tc.tile_pool(name="stg", bufs=4))
        ps_sc = ctx.enter_context(tc.tile_pool(name="ps_sc", bufs=2, space="PSUM"))
        ps_pv = ctx.enter_context(tc.tile_pool(name="ps_pv", bufs=1, space="PSUM"))
        ps_acc = ctx.enter_context(tc.tile_pool(name="ps_acc", bufs=2, space="PSUM"))

        ident = const_p.tile([128, 128], MDT)
        nc.sync.dma_start(ident[:], identd[:, :])
        # Preload the exp table set and warm the PE HAM clock gate during
        # the input-DMA window (real-hardware costs the cost model does not
        # charge; the warm matmuls also satisfy the p-state ramp).
        warm = const_p.tile([128, 8], F32)
        nc.scalar.activation(
            warm[0:1, 0:1], ident[0:1, 0:1].bitcast(F32),
            mybir.ActivationFunctionType.Exp,
        )
        warmps = ps_sc.tile([128, 2, QCH], F32, tag="sc")
        for w in range(16):
            nc.tensor.matmul(
                warmps[:, 0, 0:128], ident[:], ident[:],
                start=(w == 0), stop=(w == 15),
            )

        # ---- load inputs (chunk-0 dependencies first) --------------------
        wq_sb = big_p.tile([128, D_TILES, FH], BF16)
        wkv_sb = big_p.tile([128, D_TILES, KV], BF16)
        xt_sb = big_p.tile([128, D_TILES, S], BF16)
        nc.sync.dma_start(
            wkv_sb[:, :, :], wkv[:, :].rearrange("(a p) n -> p a n", p=128)
        )
        nc.sync.dma_start(
            wq_sb[:, :, :], wq[:, :].rearrange("(a p) n -> p a n", p=128)
        )
        for a in range(D_TILES):
            nc.sync.dma_start(xt_sb[:, a, ds(0, QCH)], xT[ds(a * 128, 128), ds(0, QCH)])
        for a in range(D_TILES):
            nc.sync.dma_start(xt_sb[:, a, ds(QCH, QCH)], xT[ds(a * 128, 128), ds(QCH, QCH)])
        for a in range(D_TILES):
            nc.sync.dma_start(
                xt_sb[:, a, ds(2 * QCH, S - 2 * QCH)],
                xT[ds(a * 128, 128), ds(2 * QCH, S - 2 * QCH)],
            )
        wo_sb = big_p.tile([128, 2, DIM], BF16)
        nc.sync.dma_start(
            wo_sb[:, :, :], wo[:, :].rearrange("(t p) n -> p t n", p=128)
        )

        # ---- projections -------------------------------------------------
        kvt_sb = big_p.tile([128, S], MDT)
        kt2_sb = big_p.tile([128, S], MDT)
        qt_sb = big_p.tile([128, 2, S], MDT)
        v1_sb = big_p.tile([128, S_TILES, 65], BF16)
        nc.vector.memset(v1_sb[:, :, 64:65], 1.0)
        ot_sb = big_p.tile([128, 2, S], BF16)

        def kvproj(sc):
            acc = ps_acc.tile([128, QCH], F32, tag="acc", name="kvacc")
            for a in range(D_TILES):
                nc.tensor.matmul(
                    acc[:],
                    wkv_sb[:, a, :],
                    xt_sb[:, a, ds(sc * QCH, QCH)],
                    start=(a == 0),
                    stop=(a == D_TILES - 1),
                )
            nc.vector.tensor_copy(kvt_sb[:, ds(sc * QCH, QCH)], acc[:])
            nc.gpsimd.dma_start(
                kt2_sb[64:128, ds(sc * QCH, QCH)], kvt_sb[0:64, ds(sc * QCH, QCH)]
            )

        def qproj(sc, fts=(0, 1)):
            for ft in fts:
                acc = ps_acc.tile([128, QCH], F32, tag="acc", name="qacc")
                for a in range(D_TILES):
                    nc.tensor.matmul(
                        acc[:],
                        wq_sb[:, a, ds(ft * 128, 128)],
                        xt_sb[:, a, ds(sc * QCH, QCH)],
                        start=(a == 0),
                        stop=(a == D_TILES - 1),
                    )
                nc.vector.tensor_copy(qt_sb[:, ft, ds(sc * QCH, QCH)], acc[:])

        def vprep(sc):
            # V^T rows of kvt (64:128) -> seq-major v1 tiles (bf16, ones col)
            for kt in range(4 * sc, 4 * sc + 4):
                tr = ps_acc.tile([128, QCH], F32, tag="acc", name="tracc")
                nc.tensor.transpose(
                    tr[:, 0:64].bitcast(MDT), kvt_sb[64:128, ds(kt * 128, 128)],
                    ident[64:128, 64:128],
                )
                nc.vector.tensor_copy(v1_sb[:, kt, 0:64], tr[:, 0:64].bitcast(MDT))

        kvproj(0)
        qproj(0, fts=(0,))
        vprep(0)
        kvproj(1)
        qproj(0, fts=(1,))
        kvproj(2)
        kvproj(3)
        for sc in range(1, N_QC):
            vprep(sc)

        # ---- output projection (per q-chunk, reads both ft of ot_sb) ----
        def outproj(qc):
            for st in range(QCH // 128):
                row0 = qc * QCH + st * 128
                for c in range(2):
                    acc = ps_acc.tile([128, 512], F32, tag="acc", name="oacc")
                    for t in range(2):
                        nc.tensor.matmul(
                            acc[:],
                            ot_sb[:, t, ds(row0, 128)],
                            wo_sb[:, t, ds(c * 512, 512)],
                            start=(t == 0),
                            stop=(t == 1),
                        )
                    stg = stg_p.tile([128, 512], BF16, tag="ostg")
                    nc.vector.tensor_copy(stg[:], acc[:])
                    nc.sync.dma_start(out[ds(row0, 128), ds(c * 512, 512)], stg[:])

        # ---- attention phases -------------------------------------------
        for qc in range(N_QC):
            qsl = ds(qc * QCH, QCH)
            for ft in range(2):
                e2t = e2_p.tile([128, 2, S_TILES, QCH], BF16, tag="e2t")
                for kt in range(S_TILES):
                    ksl = ds(kt * 128, 128)
                    sc2 = ps_sc.tile([128, 2, QCH], F32, tag="sc")
                    nc.tensor.matmul(
                        sc2[:, 0, :], kvt_sb[0:64, ksl], qt_sb[0:64, ft, qsl],
                        start=True, stop=True,
                    )
                    nc.tensor.matmul(
                        sc2[:, 1, :], kt2_sb[64:128, ksl], qt_sb[64:128, ft, qsl],
                        start=True, stop=True,
                    )
                    nc.scalar.activation(
                        e2t[:, :, kt, :], sc2[:, :, :],
                        mybir.ActivationFunctionType.Exp,
                    )

                # PV: probabilities stationary, [V|1] moving; qt-major
                # sequential accumulation groups (one open group per bank).
                of = of_p.tile([128, 2, 4, 65], F32, tag="of")
                for h in range(2):
                    pvh = ps_pv.tile([128, 4, 65], F32, tag=f"pv{h}", name=f"pv{h}")
                    for qt4 in range(4):
                        for kt in range(S_TILES):
                            nc.tensor.matmul(
                                pvh[:, qt4, :],
                                e2t[:, h, kt, ds(qt4 * 128, 128)],
                                v1_sb[:, kt, :],
                                start=(kt == 0),
                                stop=(kt == S_TILES - 1),
                            )
                    nc.vector.tensor_copy(of[:, h, :, :], pvh[:, :, :])

                # normalize (recip on DVE, scale on GPSIMD) and transpose
                o_sb = osb_p.tile([128, 4, 128], BF16, tag="osb")
                for h in range(2):
                    for qt4 in range(4):
                        rz = rz_p.tile([128, 1], F32, tag="rz", name="rz")
                        nc.vector.reciprocal(rz[:], of[:, h, qt4, 64:65])
                        nc.gpsimd.tensor_scalar(
                            o_sb[:, qt4, ds(h * 64, 64)], of[:, h, qt4, 0:64],
                            rz[:], None, AluOpType.mult,
                        )
                for qt4 in range(4):
                    nc.sync.dma_start_transpose(
                        ot_sb[:, ft, ds(qc * QCH + qt4 * 128, 128)],
                        o_sb[:, qt4, :],
                    )

                if qc + 1 < N_QC:
                    qproj(qc + 1, fts=(ft,))
                if ft == 0 and qc >= 1:
                    outproj(qc - 1)
        outproj(N_QC - 1)

    return nc


# The neuronx compiler in this environment accepts only ONE sync-wait command
# per instruction; Tile emits instructions with several. Waiting is monotone,
# so hoisting all but the last wait onto same-engine NoOps is equivalent.
_wsctr = [0]


def split_multi_waits(nc):
    n_split = 0
    for f in nc.m.functions:
        for bb in f.blocks:
            insts = bb.instructions
            if not any(
                i.sync_info is not None and len(i.sync_info.on_wait) > 1
                for i in insts
            ):
                continue
            new = []
            for i in insts:
                si = i.sync_info
                if si is not None and len(si.on_wait) > 1:
                    waits = list(si.on_wait)
                    for w in waits[:-1]:
                        _wsctr[0] += 1
                        nop = mybir.InstNoOp(name=f"wsplit_{_wsctr[0]}", ins=[], outs=[])
                        nop.engine = i.engine
                        nop.sync_info = mybir.SyncInfo(on_wait=[w], on_update=[])
                        new.append(nop)
                    i.sync_info = mybir.SyncInfo(
                        on_wait=[waits[-1]], on_update=list(si.on_update)
                    )
                    n_split += 1
                new.append(i)
            bb.instructions = new
    return n_split


def build(use_f32r=True):
    nc = bass.Bass(target_bir_lowering=False)
    build_attention_core(nc, SEQ, use_f32r=use_f32r)
    split_multi_waits(nc)
    return nc


def shard_inputs(x, wq, wk, wv, wo):
    """Full inputs -> per-core in_maps. Core c = (b = c//4, g = c%4)."""
    x = np.asarray(x, np.float32)
    wq = np.asarray(wq, np.float32)
    wk = np.asarray(wk, np.float32)
    wv = np.asarray(wv, np.float32)
    wo = np.asarray(wo, np.float32)
    ident = np.eye(128, dtype=np.float32)
    import ml_dtypes
    bf16 = ml_dtypes.bfloat16
    xTs = [np.ascontiguousarray(x[b].T).astype(bf16) for b in range(BSZ)]
    in_maps = []
    for c in range(N_CORES):
        b, g = c // 4, c % 4
        # fold the 1/sqrt(head_dim) score scaling into wq
        wq_g = (np.ascontiguousarray(wq[:, g * FH:(g + 1) * FH]) * (1.0 / np.sqrt(HD))).astype(bf16)
        wkv_g = np.ascontiguousarray(
            np.concatenate(
                [wk[:, g * HD:(g + 1) * HD], wv[:, g * HD:(g + 1) * HD]], axis=1
            )
        ).astype(bf16)
        wo_g = np.ascontiguousarray(wo[g * FH:(g + 1) * FH, :]).astype(bf16)
        in_maps.append(
            {"xT": xTs[b], "wq": wq_g, "wkv": wkv_g, "wo": wo_g,
             "ident": ident}
        )
    return in_maps


def unshard_output(results):
    """Sum the 4 row-parallel partial outputs per batch."""
    out = np.zeros((BSZ, SEQ, DIM), np.float32)
    for c in range(N_CORES):
        out[c // 4] += np.asarray(results[c]["out"], np.float32)
    return out


_cache = {}


def kernel(x, wq, wk, wv, wo):
    if "nc" not in _cache:
        _cache["nc"] = build()
    nc = _cache["nc"]
    in_maps = shard_inputs(x, wq, wk, wv, wo)
    try:
        res = bass_utils.run_bass_kernel_spmd(
            nc, in_maps, core_ids=list(range(N_CORES))
        )
    except ModuleNotFoundError:
        # BASS_TRACE under an axon client without the NTFF hook module;
        # rerun untraced.
        import os

        os.environ["BASS_NEVER_TRACE"] = "1"
        res = bass_utils.run_bass_kernel_spmd(
            nc, in_maps, core_ids=list(range(N_CORES))
        )
    return unshard_output(res.results)


# revision 4
# speedup vs baseline: 1.0902x; 1.0902x over previous
"""Distributed attention kernel for Trainium2 (8 NeuronCores).

Problem: non-causal multi-head attention with GQA (16 q heads, 4 kv heads,
head_dim 64, dim 1024, batch 2, seqlen 2048), fp32.

Sharding (per the batch+head hint): core c in 0..7 handles batch b = c//4
and kv-head-group g = c%4 (q heads 4g..4g+3, kv head g). Each core holds the
full sequence, so softmax needs no communication. The output projection is
row-parallel: core (b, g) computes the partial product
O_g @ wo[256g:256(g+1), :] and the host sums the 4 partials per batch
(the gather/unshard step).

Per-core dataflow (v2 — PV restructured to seq-major output):
  xT = x[b].T                               (1024, S) fed from host, bf16
  QT = wq_g.T @ xT                          (256, S)  f32r [head pair ft:
                                              rows 0-63 head 2ft, 64-127 2ft+1]
  KVT = [wk_g | wv_g].T @ xT                (128, S)  f32r [K^T ; V^T]
  K^T duplicated to partitions 64-127 (gpsimd DMA) so both heads of a pair
  run score matmuls from disjoint partition ranges.
  V transposed per 128-k tile (PE) and packed seq-major with a ones column:
  v1[kt] = [V_kt | 1]  (128, 65) bf16.
  Per (qc of 512 q, ft head-pair):
    per kt: S^T = K^T.T @ Q^T -> psum [128, 2, 512]; one exp (ScalarE)
            -> e2t[:, :, kt, :] bf16 (slab for the whole phase).
    PV with the probabilities STATIONARY: out[q, d] += e2^T @ [V|1]
    accumulated qt-major into [128, 4, 65] psum (sequential sub-bank
    accumulation groups; hardware allows only one OPEN group per bank).
    Cost: 65 cols/moving pass instead of 512 -> halves PV PE time.
    Normalize: DVE reciprocal of the ones-column sum, GPSIMD per-partition
    multiply -> o bf16 [q, 2*64]; XBAR dma transpose -> ot feature-major.
  out rows = OT.T @ wo_g (PE), staged bf16, DMA out. Host sums partials.

Engine budget (cost model): ACT exp 128x~1.04us = 133us (bound), PE ~287K
cyc = 120us, DVE ~43us, Pool ~12us, DMA ~28us.
"""

import numpy as np
from contextlib import ExitStack

import concourse.bass as bass
import concourse.mybir as mybir
import concourse.tile as tile
from concourse.bass import ds
from concourse import bass_utils
from concourse.alu_op_type import AluOpType

F32 = mybir.dt.float32
F32R = mybir.dt.float32r
BF16 = mybir.dt.bfloat16

DIM = 1024
N_HEADS = 16
N_KV_HEADS = 4
HD = 64
FH = 256                   # q features per core (4 heads x 64)
KV = 128                   # [K | V] projected feature width per core
D_TILES = DIM // 128       # 8
SEQ = 2048
BSZ = 2
N_CORES = 8


def build_attention_core(nc, S=SEQ, use_f32r=True):
    """Emit the per-core kernel. S = sequence length (multiple of 512)."""
    QCH = 512                  # q-chunk width (psum bank = 512 f32)
    S_TILES = S // 128         # k tiles
    N_QC = S // QCH            # q chunks
    MDT = F32R if use_f32r else F32

    xT = nc.declare_dram_parameter("xT", [DIM, S], BF16, isOutput=False)
    identd = nc.declare_dram_parameter("ident", [128, 128], MDT, isOutput=False)
    wq = nc.declare_dram_parameter("wq", [DIM, FH], BF16, isOutput=False)
    wkv = nc.declare_dram_parameter("wkv", [DIM, KV], BF16, isOutput=False)
    wo = nc.declare_dram_parameter("wo", [FH, DIM], BF16, isOutput=False)
    out = nc.declare_dram_parameter("out", [S, DIM], BF16, isOutput=True)

    with tile.TileContext(nc) as tc:
      with ExitStack() as ctx:
        const_p = ctx.enter_context(tc.tile_pool(name="const", bufs=1))
        big_p = ctx.enter_context(tc.tile_pool(name="big", bufs=1))
        e2_p = ctx.enter_context(tc.tile_pool(name="e2", bufs=2))
        of_p = ctx.enter_context(tc.tile_pool(name="of", bufs=2))
        osb_p = ctx.enter_context(tc.tile_pool(name="osb", bufs=2))
        rz_p = ctx.enter_context(tc.tile_pool(name="rz", bufs=8))
        stg_p = ctx.enter_context(tc.tile_pool(name="stg", bufs=4))
        ps_sc = ctx.enter_context(tc.tile_pool(name="ps_sc", bufs=2, space="PSUM"))
        ps_pv = ctx.enter_context(tc.tile_pool(name="ps_pv", bufs=1, space="PSUM"))
        ps_acc = ctx.enter_context(tc.tile_pool(name="ps_acc", bufs=2, space="PSUM"))

        ident = const_p.tile([128, 128], MDT)
        nc.sync.dma_start(ident[:], identd[:, :])
        # Preload the exp table set and warm the PE HAM clock gate during
        # the input-DMA window (real-hardware costs the cost model does not
        # charge; the warm matmuls also satisfy the p-state ramp).
        warm = const_p.tile([128, 8], F32)
        nc.scalar.activation(
            warm[0:1, 0:1], ident[0:1, 0:1].bitcast(F32),
            mybir.ActivationFunctionType.Exp,
        )
        warmps = ps_sc.tile([128, 2, QCH], F32, tag="sc")
        for w in range(16):
            nc.tensor.matmul(
                warmps[:, 0, 0:128], ident[:], ident[:],
                start=(w == 0), stop=(w == 15),
            )

        # ---- load inputs, ordered so kvproj chunks land just-in-time -----
        wq_sb = big_p.tile([128, D_TILES, FH], BF16)
        wkv_sb = big_p.tile([128, D_TILES, KV], BF16)
        xt_sb = big_p.tile([128, D_TILES, S], BF16)
        nc.sync.dma_start(
            wkv_sb[:, :, :], wkv[:, :].rearrange("(a p) n -> p a n", p=128)
        )
        for a in range(D_TILES):
            nc.sync.dma_start(xt_sb[:, a, ds(0, QCH)], xT[ds(a * 128, 128), ds(0, QCH)])
        nc.sync.dma_start(
            wq_sb[:, :, :], wq[:, :].rearrange("(a p) n -> p a n", p=128)
        )
        for sc in range(1, N_QC):
            for a in range(D_TILES):
                nc.sync.dma_start(
                    xt_sb[:, a, ds(sc * QCH, QCH)],
                    xT[ds(a * 128, 128), ds(sc * QCH, QCH)],
                )
        wo_sb = big_p.tile([128, 2, DIM], BF16)
        nc.sync.dma_start(
            wo_sb[:, :, :], wo[:, :].rearrange("(t p) n -> p t n", p=128)
        )

        # ---- projections -------------------------------------------------
        kvt_sb = big_p.tile([128, S], MDT)
        kt2_sb = big_p.tile([128, S], MDT)
        qt_sb = big_p.tile([128, 2, S], MDT)
        v1_sb = big_p.tile([128, S_TILES, 65], BF16)
        nc.vector.memset(v1_sb[:, :, 64:65], 1.0)
        ot_sb = big_p.tile([128, 2, S], BF16)

        def kvproj(sc):
            acc = ps_acc.tile([128, QCH], F32, tag="acc", name="kvacc")
            for a in range(D_TILES):
                nc.tensor.matmul(
                    acc[:],
                    wkv_sb[:, a, :],
                    xt_sb[:, a, ds(sc * QCH, QCH)],
                    start=(a == 0),
                    stop=(a == D_TILES - 1),
                )
            nc.vector.tensor_copy(kvt_sb[:, ds(sc * QCH, QCH)], acc[:])
            nc.gpsimd.dma_start(
                kt2_sb[64:128, ds(sc * QCH, QCH)], kvt_sb[0:64, ds(sc * QCH, QCH)]
            )

        def qproj(sc, fts=(0, 1)):
            for ft in fts:
                acc = ps_acc.tile([128, QCH], F32, tag="acc", name="qacc")
                for a in range(D_TILES):
                    nc.tensor.matmul(
                        acc[:],
                        wq_sb[:, a, ds(ft * 128, 128)],
                        xt_sb[:, a, ds(sc * QCH, QCH)],
                        start=(a == 0),
                        stop=(a == D_TILES - 1),
                    )
                nc.vector.tensor_copy(qt_sb[:, ft, ds(sc * QCH, QCH)], acc[:])

        def vprep(sc):
            # V^T rows of kvt (64:128) -> seq-major v1 tiles (bf16, ones col)
            for kt in range(4 * sc, 4 * sc + 4):
                tr = ps_acc.tile([128, QCH], F32, tag="acc", name="tracc")
                nc.tensor.transpose(
                    tr[:, 0:64].bitcast(MDT), kvt_sb[64:128, ds(kt * 128, 128)],
                    ident[64:128, 64:128],
                )
                nc.vector.tensor_copy(v1_sb[:, kt, 0:64], tr[:, 0:64].bitcast(MDT))

        kvproj(0)
        qproj(0, fts=(0,))
        kvproj(1)
        kvproj(2)
        kvproj(3)
        qproj(0, fts=(1,))
        for sc in range(N_QC):
            vprep(sc)

        # ---- deferred-work generators -----------------------------------
        def outproj_block(qc, st, c):
            row0 = qc * QCH + st * 128
            acc = ps_acc.tile([128, 512], F32, tag="acc", name="oacc")
            for t in range(2):
                nc.tensor.matmul(
                    acc[:],
                    ot_sb[:, t, ds(row0, 128)],
                    wo_sb[:, t, ds(c * 512, 512)],
                    start=(t == 0),
                    stop=(t == 1),
                )
            stg = stg_p.tile([128, 512], BF16, tag="ostg", name="stg")
            nc.vector.tensor_copy(stg[:], acc[:])
            nc.sync.dma_start(out[ds(row0, 128), ds(c * 512, 512)], stg[:])

        def outproj_gen(qc):
            for st in range(QCH // 128):
                outproj_block(qc, st, 0)
                outproj_block(qc, st, 1)
                yield

        def pv_work(qc, ft, e2t, tail=False):
            """PV + normalize + transpose for phase (qc, ft); yields between
            slices so the caller can interleave it with the next phase."""
            of = of_p.tile([128, 2, 4, 65], F32, tag="of", name="of")
            o_sb = osb_p.tile([128, 4, 128], BF16, tag="osb", name="osb")
            for h in range(2):
                pvh = ps_pv.tile([128, 4, 65], F32, tag=f"pv{h}", name=f"pv{h}")
                for qt4 in range(4):
                    for kt in range(S_TILES):
                        nc.tensor.matmul(
                            pvh[:, qt4, :],
                            e2t[:, h, kt, ds(qt4 * 128, 128)],
                            v1_sb[:, kt, :],
                            start=(kt == 0),
                            stop=(kt == S_TILES - 1),
                        )
                    yield
                nc.vector.tensor_copy(of[:, h, :, :], pvh[:, :, :])
                for qt4 in range(4):
                    rz = rz_p.tile([128, 1], F32, tag="rz", name="rz")
                    nc.vector.reciprocal(rz[:], of[:, h, qt4, 64:65])
                    nc.gpsimd.tensor_scalar(
                        o_sb[:, qt4, ds(h * 64, 64)], of[:, h, qt4, 0:64],
                        rz[:], None, AluOpType.mult,
                    )
                yield
            for qt4 in range(4):
                nc.sync.dma_start_transpose(
                    ot_sb[:, ft, ds(qc * QCH + qt4 * 128, 128)],
                    o_sb[:, qt4, :],
                )
                if tail:
                    # final phase: chase each transpose with its out rows
                    outproj_block(qc, qt4, 0)
                    outproj_block(qc, qt4, 1)
                yield

        # ---- attention phases -------------------------------------------
        prev_gen = None
        for qc in range(N_QC):
            qsl = ds(qc * QCH, QCH)
            for ft in range(2):
                e2t = e2_p.tile([128, 2, S_TILES, QCH], BF16, tag="e2t",
                                name="e2t")
                # deferred work interleaved into this phase's kt slots:
                # previous phase's PV chain, this-column qproj, and the
                # output projection two phases back (at ft==1).
                steps = []
                if qc + 1 < N_QC:
                    steps.append(lambda qc=qc, ft=ft: qproj(qc + 1, fts=(ft,)))
                gens = [g for g in [prev_gen] if g is not None]
                if ft == 1 and qc >= 1:
                    gens.append(outproj_gen(qc - 1))

                def drain_one():
                    while gens:
                        try:
                            next(gens[0])
                            return
                        except StopIteration:
                            gens.pop(0)

                for kt in range(S_TILES):
                    ksl = ds(kt * 128, 128)
                    sc2 = ps_sc.tile([128, 2, QCH], F32, tag="sc", name="sc2")
                    nc.tensor.matmul(
                        sc2[:, 0, :], kvt_sb[0:64, ksl], qt_sb[0:64, ft, qsl],
                        start=True, stop=True,
                    )
                    nc.tensor.matmul(
                        sc2[:, 1, :], kt2_sb[64:128, ksl], qt_sb[64:128, ft, qsl],
                        start=True, stop=True,
                    )
                    nc.scalar.activation(
                        e2t[:, :, kt, :], sc2[:, :, :],
                        mybir.ActivationFunctionType.Exp,
                    )
                    if steps:
                        steps.pop(0)()
                    else:
                        drain_one()
                while gens:
                    drain_one()

                last = (qc == N_QC - 1 and ft == 1)
                prev_gen = pv_work(qc, ft, e2t, tail=last)
        # tail: final phase's PV chain + its output projection
        for _ in prev_gen:
            pass

    return nc


# The neuronx compiler in this environment accepts only ONE sync-wait command
# per instruction; Tile emits instructions with several. Waiting is monotone,
# so hoisting all but the last wait onto same-engine NoOps is equivalent.
_wsctr = [0]


def split_multi_waits(nc):
    n_split = 0
    for f in nc.m.functions:
        for bb in f.blocks:
            insts = bb.instructions
            if not any(
                i.sync_info is not None and len(i.sync_info.on_wait) > 1
                for i in insts
            ):
                continue
            new = []
            for i in insts:
                si = i.sync_info
                if si is not None and len(si.on_wait) > 1:
                    waits = list(si.on_wait)
                    for w in waits[:-1]:
                        _wsctr[0] += 1
                        nop = mybir.InstNoOp(name=f"wsplit_{_wsctr[0]}", ins=[], outs=[])
                        nop.engine = i.engine
                        nop.sync_info = mybir.SyncInfo(on_wait=[w], on_update=[])
                        new.append(nop)
                    i.sync_info = mybir.SyncInfo(
                        on_wait=[waits[-1]], on_update=list(si.on_update)
                    )
                    n_split += 1
                new.append(i)
            bb.instructions = new
    return n_split


def build(use_f32r=True):
    nc = bass.Bass(target_bir_lowering=False)
    build_attention_core(nc, SEQ, use_f32r=use_f32r)
    split_multi_waits(nc)
    return nc


def shard_inputs(x, wq, wk, wv, wo):
    """Full inputs -> per-core in_maps. Core c = (b = c//4, g = c%4)."""
    x = np.asarray(x, np.float32)
    wq = np.asarray(wq, np.float32)
    wk = np.asarray(wk, np.float32)
    wv = np.asarray(wv, np.float32)
    wo = np.asarray(wo, np.float32)
    ident = np.eye(128, dtype=np.float32)
    import ml_dtypes
    bf16 = ml_dtypes.bfloat16
    xTs = [np.ascontiguousarray(x[b].T).astype(bf16) for b in range(BSZ)]
    in_maps = []
    for c in range(N_CORES):
        b, g = c // 4, c % 4
        # fold the 1/sqrt(head_dim) score scaling into wq
        wq_g = (np.ascontiguousarray(wq[:, g * FH:(g + 1) * FH]) * (1.0 / np.sqrt(HD))).astype(bf16)
        wkv_g = np.ascontiguousarray(
            np.concatenate(
                [wk[:, g * HD:(g + 1) * HD], wv[:, g * HD:(g + 1) * HD]], axis=1
            )
        ).astype(bf16)
        wo_g = np.ascontiguousarray(wo[g * FH:(g + 1) * FH, :]).astype(bf16)
        in_maps.append(
            {"xT": xTs[b], "wq": wq_g, "wkv": wkv_g, "wo": wo_g,
             "ident": ident}
        )
    return in_maps


def unshard_output(results):
    """Sum the 4 row-parallel partial outputs per batch."""
    out = np.zeros((BSZ, SEQ, DIM), np.float32)
    for c in range(N_CORES):
        out[c // 4] += np.asarray(results[c]["out"], np.float32)
    return out


_cache = {}


def kernel(x, wq, wk, wv, wo):
    if "nc" not in _cache:
        _cache["nc"] = build()
    nc = _cache["nc"]
    in_maps = shard_inputs(x, wq, wk, wv, wo)
    try:
        res = bass_utils.run_bass_kernel_spmd(
            nc, in_maps, core_ids=list(range(N_CORES))
        )
    except ModuleNotFoundError:
        # BASS_TRACE under an axon client without the NTFF hook module;
        # rerun untraced.
        import os

        os.environ["BASS_NEVER_TRACE"] = "1"
        res = bass_utils.run_bass_kernel_spmd(
            nc, in_maps, core_ids=list(range(N_CORES))
        )
    return unshard_output(res.results)
